# revision 1
# baseline (speedup 1.0000x reference)
"""Trainium2 Bass kernel for nn_GAT_39427799777563 (GAT message passing).

Math (per item row n, K=32 neighbors, D=100 dims):
    We   = entity_embs * w_r                  # [K, D] elementwise
    e_k  = sum_d We[k, d]                     # neighbor logits
    p_k  = adj_k * exp(leaky_relu(e_k))       # masked unnormalized softmax
    h'   = sum_k p_k * We[k, :]               # weighted neighbor sum (unnorm.)
    x    = (h' @ W_out.T) / sum_k p_k + (b_out + item_embs)

Sparsity packing: adj masks ~half the (n, k) pairs, and masked neighbors
contribute nothing to the output (p_k = 0 regardless of e_k). The host
therefore packs each row's active k's to the front (per-row gather of
ent/wr/adj along k), sorts rows by active count, and groups them into
256-row pairs. Each pair only loads/computes its own K_p <= 32 packed
neighbors (~16.5 on average) -- halving both HBM traffic and the
K-proportional DVE/ACT work, losslessly in fp32. Entries beyond a row's
count hold that row's *masked* neighbors; the packed adj mask zeroes them
exactly as before, so the math is bit-identical to the unpacked kernel.

SPMD note: all 8 cores run one program, so the 160 sorted pairs are
striped across cores (global pair 8j+c -> core c, slot j) and slot j's
K is the max over its 8 cores -- adjacent sorted pairs, so the waste is
<= 1-2 k's. Rows are un-permuted on the host after the gather.

The max-subtraction in the reference softmax is dropped: e is bounded
so exp(e) stays inside fp32 range, and softmax is shift-invariant.
Masking happens after exp. The 1/denominator scale and the bias+residual
add are folded into one scalar_tensor_tensor on the matmul output
(item_embs + b_out is precomputed on host).

Engine split per 256-row pair (fp32 everywhere):
    DVE : We mul, min(8, K_p) of the e-sums per tile (strided reduce),
          leaky-relu, mask, denominator, reciprocal, p-broadcast
          weighting mul (in place), strided k-reduction, fused
          scale+residual epilogue
    ACT : remaining e-sums via activation(Copy, accum_out), exp,
          PSUM->SBUF copy of transposed h'
    PE  : h' transpose + the 100x100 linear
    SP  : all DMA (HWDGE)
"""

from contextlib import ExitStack

import numpy as np

import concourse.bass as bass
import concourse.bacc as bacc
import concourse.mybir as mybir
import concourse.tile as tile

F32 = mybir.dt.float32
ALPHA = 0.2

N, K, D = 40000, 32, 100
N_CORES = 8
P = 128            # rows per tile == SBUF partitions
J = 2              # tiles per pair
import os as _os
M_DVE = int(_os.environ.get("GAT_M_DVE", "8"))  # k's d-summed on DVE; rest ACT
STORE_CHUNK = 8    # tiles per output store
_N_TILES_FULL = 40  # 8 cores * 40 tiles * 128 rows = 40960 >= 40000


def build(n_tiles: int, repeats: int = 1, mode: str = "full", klist=None):
    """Build the per-core Bass program.

    klist: per-pair packed-K values (len n_tiles/2); None -> dense K=32.
    repeats > 1 wraps the tile loop in a hardware For_i loop (for
    dispatch-overhead-free benchmarking)."""
    if klist is None:
        klist = [K] * (n_tiles // J)
    assert len(klist) == n_tiles // J
    rows = n_tiles * P
    tot = sum(J * P * kp * D for kp in klist)
    sum2k = sum(J * kp for kp in klist)

    nc = bacc.Bacc("TRN2", target_bir_lowering=False, debug=False,
                   num_devices=N_CORES)

    ent_d = nc.dram_tensor("ent", [tot], F32, kind="ExternalInput")
    wr_d = nc.dram_tensor("wr", [tot], F32, kind="ExternalInput")
    adj_d = nc.dram_tensor("adjf", [P, sum2k], F32, kind="ExternalInput")
    itemb_d = nc.dram_tensor("itemb", [P, n_tiles * D], F32, kind="ExternalInput")
    wt_d = nc.dram_tensor("wt", [D, D], F32, kind="ExternalInput")   # W_out.T
    ident_d = nc.dram_tensor("ident", [P, P], F32, kind="ExternalInput")
    out_d = nc.dram_tensor("out", [P, n_tiles * D], F32, kind="ExternalOutput")

    kmax = max(klist)

    with tile.TileContext(nc) as tc, ExitStack() as ctx:
        const = ctx.enter_context(tc.tile_pool(name="const", bufs=1))
        big = ctx.enter_context(tc.tile_pool(name="big", bufs=2))
        wep = ctx.enter_context(tc.tile_pool(name="wep", bufs=2))
        small = ctx.enter_context(tc.tile_pool(name="small", bufs=2))
        psum = ctx.enter_context(tc.tile_pool(name="psum", bufs=2, space="PSUM"))

        adjf = const.tile([P, sum2k], F32)
        itemb = const.tile([P, n_tiles * D], F32)
        wt = const.tile([D, D], F32)
        ident = const.tile([P, P], F32)
        out_all = const.tile([P, n_tiles * D], F32)
        nc.sync.dma_start(adjf[:], adj_d[:])
        nc.sync.dma_start(itemb[:], itemb_d[:])
        nc.sync.dma_start(wt[:], wt_d[:])
        nc.sync.dma_start(ident[:], ident_d[:])

        def tile_loop():
            body_pairs(nc, n_tiles, klist, kmax, ent_d, wr_d, out_d, adjf,
                       itemb, wt, ident, out_all, big, wep, small, psum, mode)

        if repeats > 1:
            with tc.For_i(0, repeats, 1):
                tile_loop()
        else:
            tile_loop()

    nc.compile()
    return nc


def body_pairs(nc, n_tiles, klist, kmax, ent_d, wr_d, out_d, adjf, itemb,
               wt, ident, out_all, big, wep, small, psum, mode):
    AF = mybir.ActivationFunctionType
    AL = mybir.AluOpType
    AX = mybir.AxisListType

    off = 0    # element offset into the packed ent/wr buffers
    aoff = 0   # column offset into the packed adj mask
    for pg in range(n_tiles // J):
        kp = klist[pg]
        kf = kp * D
        blk = J * P * kf

        ent_t = big.tile([P, J * kmax * D], F32, tag="ent")
        nc.sync.dma_start(
            ent_t[:, :J * kf].rearrange("p (j f) -> p j f", j=J),
            ent_d[off:off + blk].rearrange("(j p f) -> p j f", j=J, p=P))
        wr_t = big.tile([P, J * kmax * D], F32, tag="wr")
        nc.sync.dma_start(
            wr_t[:, :J * kf].rearrange("p (j f) -> p j f", j=J),
            wr_d[off:off + blk].rearrange("(j p f) -> p j f", j=J, p=P))

        if mode == "dma":
            for j in range(J):
                t = pg * J + j
                nc.vector.tensor_copy(out_all[:, t * D:(t + 1) * D],
                                      ent_t[:, j * kf:j * kf + D])
            if (pg + 1) % (STORE_CHUNK // J) == 0:
                csl = slice((pg + 1 - STORE_CHUNK // J) * J * D,
                            (pg + 1) * J * D)
                nc.sync.dma_start(out_d[:, csl], out_all[:, csl])
            off += blk
            aoff += J * kp
            continue

        # We = ent * wr   (DVE, one big 1x pass)
        we = wep.tile([P, J * kmax * D], F32, tag="we")
        nc.vector.tensor_mul(we[:, :J * kf], ent_t[:, :J * kf],
                             wr_t[:, :J * kf])

        # e_{j,k} = sum_d We[j, k, :]: first m k's per tile on DVE (one
        # strided reduce), the rest on ACT (accumulate, in-place copy)
        m = min(M_DVE, kp)
        e = small.tile([P, J * kmax], F32, tag="e")
        if m > 0:
            nc.vector.tensor_reduce(
                e[:, :J * kp].rearrange("p (j k) -> p j k", j=J)[:, :, :m],
                we[:, :J * kf].rearrange("p (j k d) -> p j k d",
                                         j=J, k=kp)[:, :, :m, :],
                axis=AX.X, op=AL.add,
            )
        for j in range(J):
            for k in range(m, kp):
                ksl = slice(j * kf + k * D, j * kf + (k + 1) * D)
                nc.scalar.activation(we[:, ksl], we[:, ksl], AF.Copy,
                                     accum_out=e[:, j * kp + k:j * kp + k + 1])

        # leaky relu (DVE): elr = max(alpha*e, e)
        elr = small.tile([P, J * kmax], F32, tag="elr")
        nc.vector.scalar_tensor_tensor(elr[:, :J * kp], e[:, :J * kp],
                                       ALPHA, e[:, :J * kp],
                                       op0=AL.mult, op1=AL.max)
        # exp (ACT)
        ex = small.tile([P, J * kmax], F32, tag="ex")
        nc.scalar.activation(ex[:, :J * kp], elr[:, :J * kp], AF.Exp)

        # p = ex * adj ; sumexp = sum_k p ; rs = 1/sumexp
        p = small.tile([P, J * kmax], F32, tag="p")
        nc.vector.tensor_mul(p[:, :J * kp], ex[:, :J * kp],
                             adjf[:, aoff:aoff + J * kp])
        sumexp = small.tile([P, J], F32, tag="sumexp")
        nc.vector.tensor_reduce(
            sumexp[:], p[:, :J * kp].rearrange("p (j k) -> p j k", j=J),
            axis=AX.X, op=AL.add)
        rs = small.tile([P, J], F32, tag="rs")
        nc.vector.reciprocal(rs[:], sumexp[:])

        # We *= p (k-broadcast over d), in place (DVE)
        we3 = we[:, :J * kf].rearrange("p (j k d) -> p j k d", j=J, k=kp)
        p3 = (p[:, :J * kp].rearrange("p (j k) -> p j k", j=J)
              .unsqueeze(-1).broadcast_to([P, J, kp, D]))
        nc.vector.tensor_mul(we3, we3, p3)

        # h'_u[j, d] = sum_k We[j, k, d]  (DVE strided reduce, innermost=k)
        hu = small.tile([P, J * D], F32, tag="hu")
        nc.vector.tensor_reduce(
            hu[:].rearrange("p (j d) -> p j d", j=J),
            we[:, :J * kf].rearrange("p (j k d) -> p j d k", j=J, k=kp),
            axis=AX.X, op=AL.add,
        )

        for j in range(J):
            t = pg * J + j
            # transpose h' -> [D, P] (PE), copy PSUM->SBUF (ACT)
            ht_ps = psum.tile([D, P], F32, tag="htp")
            nc.tensor.transpose(ht_ps[:], hu[:, j * D:(j + 1) * D], ident[:])
            ht = small.tile([D, P], F32, tag="ht")
            nc.scalar.copy(ht[:], ht_ps[:])
            # x_mm = h'_u @ W_out.T  (PE)
            x_ps = psum.tile([P, D], F32, tag="xps")
            nc.tensor.matmul(x_ps[:], ht[:], wt[:], start=True, stop=True)
            # out = x_mm * (1/sumexp) + (item + b)  (DVE fused epilogue)
            nc.vector.scalar_tensor_tensor(
                out_all[:, t * D:(t + 1) * D], x_ps[:], rs[:, j:j + 1],
                itemb[:, t * D:(t + 1) * D], op0=AL.mult, op1=AL.add,
            )

        if (pg + 1) % (STORE_CHUNK // J) == 0:
            csl = slice((pg + 1 - STORE_CHUNK // J) * J * D,
                        (pg + 1) * J * D)
            nc.sync.dma_start(out_d[:, csl], out_all[:, csl])

        off += blk
        aoff += J * kp

    n_pairs = n_tiles // J
    rem = n_pairs % (STORE_CHUNK // J)
    if rem:
        csl = slice((n_pairs - rem) * J * D, n_pairs * J * D)
        nc.sync.dma_start(out_d[:, csl], out_all[:, csl])


def _shard_host(item_embs, entity_embs, w_r, adj, W_out, b_out, n_tiles):
    """Sort rows by active-neighbor count, pack active k's to the front,
    stripe sorted 256-row pairs across cores, and build the per-core
    ragged input buffers. Returns (in_maps, klist, order)."""
    rows = n_tiles * P
    n_pad = N_CORES * rows
    n_pairs = n_tiles // J

    ent = np.asarray(entity_embs, np.float32).reshape(N, K, D)
    wr = np.asarray(w_r, np.float32).reshape(N, K, D)
    adjf = np.asarray(adj).astype(np.float32)
    itemb = np.asarray(item_embs, np.float32) + np.asarray(b_out, np.float32)

    pad = n_pad - N
    ent = np.pad(ent, ((0, pad), (0, 0), (0, 0)))
    wr = np.pad(wr, ((0, pad), (0, 0), (0, 0)))
    # padding rows: one active (zero) neighbor -> nonzero denominator,
    # count 1 so they sort to the sparse end
    adjp = np.pad(adjf, ((0, pad), (0, 0)))
    adjp[N:, 0] = 1.0
    itemb = np.pad(itemb, ((0, pad), (0, 0)))

    counts = adjp.sum(1).astype(np.int64)
    order = np.argsort(counts, kind="stable")

    # global sorted pairs of 256 rows; slot j = pairs 8j..8j+7 (one per core)
    pair_k = counts[order].reshape(-1, J * P).max(1)       # [cores * n_pairs]
    klist = [int(pair_k[8 * j: 8 * j + 8].max()) for j in range(n_pairs)]

    # pack active k's first (stable: keeps original k order)
    ai_full = np.argsort(1.0 - adjp, axis=1, kind="stable")  # [n_pad, K]

    wt = np.ascontiguousarray(np.asarray(W_out, np.float32).T)
    ident = np.eye(P, dtype=np.float32)

    in_maps = []
    for c in range(N_CORES):
        ent_parts, wr_parts, adj_sw = [], [], []
        it_sw = np.empty((P, n_tiles * D), np.float32)
        for j in range(n_pairs):
            g = 8 * j + c
            rsel = order[g * J * P:(g + 1) * J * P]
            kp = klist[j]
            ai = ai_full[rsel, :kp]
            ent_parts.append(
                np.take_along_axis(ent[rsel], ai[:, :, None], 1).ravel())
            wr_parts.append(
                np.take_along_axis(wr[rsel], ai[:, :, None], 1).ravel())
            a_p = np.take_along_axis(adjp[rsel], ai, 1)      # [256, kp]
            adj_sw.append(a_p.reshape(J, P, kp).transpose(1, 0, 2)
                          .reshape(P, J * kp))
            it = itemb[rsel].reshape(J, P, D).transpose(1, 0, 2)
            it_sw[:, j * J * D:(j + 1) * J * D] = it.reshape(P, J * D)
        in_maps.append({
            "ent": np.concatenate(ent_parts),
            "wr": np.concatenate(wr_parts),
            "adjf": np.ascontiguousarray(np.concatenate(adj_sw, axis=1)),
            "itemb": it_sw,
            "wt": wt,
            "ident": ident,
        })
    return in_maps, klist, order


def _unshard_host(results, n_tiles, order):
    n_pairs = n_tiles // J
    res_sorted = np.empty((N_CORES * n_tiles * P, D), np.float32)
    for c in range(N_CORES):
        o = results[c]["out"]  # [P, n_tiles * D]
        for j in range(n_pairs):
            g = 8 * j + c
            blk = (o[:, j * J * D:(j + 1) * J * D]
                   .reshape(P, J, D).transpose(1, 0, 2).reshape(J * P, D))
            res_sorted[g * J * P:(g + 1) * J * P] = blk
    out = np.empty_like(res_sorted)
    out[order] = res_sorted
    return out[:N]


def kernel(item_embs, entity_embs, w_r, adj, W_out, b_out):
    from concourse.bass_utils import run_bass_kernel_spmd

    in_maps, klist, order = _shard_host(item_embs, entity_embs, w_r, adj,
                                        W_out, b_out, _N_TILES_FULL)
    nc = build(_N_TILES_FULL, klist=klist)
    res = run_bass_kernel_spmd(nc, in_maps, core_ids=list(range(N_CORES)))
    return _unshard_host(res.results, _N_TILES_FULL, order).astype(np.float32)

